# revision 14
# baseline (speedup 1.0000x reference)
"""Trainium2 Bass kernel: 2-layer LSTM decoder with embedding lookup.

Reference computation (per nn.Decoder):
    tgt_embed = emb[prev_tgt_tokens]                      # [B, T, D]
    for t in 0..T-1:
        x = tgt_embed[:, t]
        for l in 0..1:
            gates = x @ W_ih[l].T + b_ih[l] + h[l] @ W_hh[l].T + b_hh[l]
            i, f, g, o = split(gates, 4)
            c[l] = sigmoid(f) * c[l] + sigmoid(i) * tanh(g)
            h[l] = sigmoid(o) * tanh(c[l])
            x = h[l]
        out[:, t] = h[1]

Sharding: data-parallel over batch B=64 across 8 cores (8 rows each);
embedding + LSTM weights replicated; the sequential time loop runs
on-device per core, fully unrolled.

Kernel design (per core):
  - All matmul operands (emb rows, weights, h) are bf16, converted
    host-side so no on-device staging/convert copies are needed and the
    HBM weight traffic is halved. PSUM accumulation and the gate/state
    chain stay fp32.
  - Embedding gather via indirect DMA (128 rows per call), PE-transposed
    into K-major layout.
  - Input projection x @ W_ih[0].T batched over all T steps as one big
    matmul; the result stays resident in SBUF (bf16) instead of a DRAM
    round trip, and is staged per step with a small SBUF->SBUF DMA.
  - Recurrent loop: iteration t runs layer-0 step t and layer-1 step t-1.
    Each lane has its own PSUM tile and its own (unstacked) activation
    chain so the lane-0 recurrence - which gates the next iteration's
    matmuls - completes while the PE is still streaming lane-1 weights.
    The lane-1 h-transpose is deferred one further iteration so it never
    head-of-line blocks the PE queue.
  - All activations are Sigmoid (tanh(x) = 2*sigmoid(2x)-1) so the ACT
    engine never reloads its function table.
"""

import os

import numpy as np

import concourse.bass as bass
import concourse.mybir as mybir
import concourse.tile as tile
from concourse import bacc
from concourse.bass_utils import run_bass_kernel_spmd
from concourse.masks import make_identity

N_CORES = 8
B = 64
T = int(os.environ.get("BASS_LSTM_T", "128"))
D = 512
V = 32000
G = 4 * D            # 2048 gate dims per layer
BL = B // N_CORES    # 8 batch rows per core
KC = D // 128        # 4 contraction chunks of 128
NB = G // 512        # 4 PSUM banks of 512 per gate vector
MT = BL * T // 128   # M-tiles (128 token rows each) for the input matmul
TPM = 128 // BL      # time steps per M-tile (16)
REPS = int(os.environ.get("BASS_LSTM_REPS", "1"))  # timing-only: loop phase B
F32 = mybir.dt.float32
BF16 = mybir.dt.bfloat16
I32 = mybir.dt.int32
AFT = mybir.ActivationFunctionType

# gate banks after host-side permutation: [f, i, g, o]
BANK_F, BANK_I, BANK_G, BANK_O = 0, 1, 2, 3
FI, GSL, OSL = slice(0, 1024), slice(1024, 1536), slice(1536, 2048)


def _nsl(n):
    return slice(n * 512, (n + 1) * 512)


def _build():
    nc = bacc.Bacc(
        "TRN2",
        target_bir_lowering=False,
        debug=False,
        enable_asserts=False,
        num_devices=N_CORES,
    )

    tok_d = nc.dram_tensor("tokens", [BL * T, 1], I32, kind="ExternalInput")
    emb_d = nc.dram_tensor("emb", [V, D], BF16, kind="ExternalInput")
    wih0_d = nc.dram_tensor("wih0t", [D, G], BF16, kind="ExternalInput")
    whh0_d = nc.dram_tensor("whh0t", [D, G], BF16, kind="ExternalInput")
    wih1_d = nc.dram_tensor("wih1t", [D, G], BF16, kind="ExternalInput")
    whh1_d = nc.dram_tensor("whh1t", [D, G], BF16, kind="ExternalInput")
    bias0_d = nc.dram_tensor("bias0", [128, G], BF16, kind="ExternalInput")
    bias1_d = nc.dram_tensor("bias1", [BL, G], F32, kind="ExternalInput")
    ht_d = nc.dram_tensor("ht_init", [2, 128, KC * BL], BF16, kind="ExternalInput")
    c_d = nc.dram_tensor("c_init", [2, BL, D], F32, kind="ExternalInput")
    out_d = nc.dram_tensor("out", [BL, T, D], F32, kind="ExternalOutput")

    with tile.TileContext(nc) as tc:
        _body(
            tc,
            tok=tok_d.ap(),
            emb=emb_d.ap(),
            w=[wih0_d.ap(), whh0_d.ap(), wih1_d.ap(), whh1_d.ap()],
            bias0=bias0_d.ap(),
            bias1=bias1_d.ap(),
            ht0=ht_d.ap(),
            c0=c_d.ap(),
            out=out_d.ap(),
        )
    nc.compile()
    return nc


def _body(tc, tok, emb, w, bias0, bias1, ht0, c0, out):
    nc = tc.nc
    with (
        tc.tile_pool(name="wpool", bufs=1) as wp,
        tc.tile_pool(name="state", bufs=1) as st,
        tc.tile_pool(name="work", bufs=2) as wk,
        tc.tile_pool(name="pspool", bufs=1, space="PSUM") as pp,
    ):
        # ---- persistent tiles -------------------------------------------
        id_sb = wp.tile([128, 128], F32)
        make_identity(nc, id_sb[:])
        id_bf = wp.tile([128, 128], BF16)
        make_identity(nc, id_bf[:])

        whh0_sb = wp.tile([128, KC * G], BF16)
        wih1_sb = wp.tile([128, KC * G], BF16)
        whh1_sb = wp.tile([128, KC * G], BF16)

        def load_w(dst, src_ap):
            # one DMA: [D, G] viewed as [128, KC, G] chunk-major
            nc.sync.dma_start(
                out=dst[:].rearrange("p (c n) -> p c n", c=KC),
                in_=src_ap.rearrange("(c p) n -> p c n", p=128),
            )

        load_w(whh0_sb, w[1])
        load_w(wih1_sb, w[2])
        load_w(whh1_sb, w[3])

        bias1_sb = wp.tile([BL, G], F32)
        nc.sync.dma_start(out=bias1_sb[:], in_=bias1)

        # input projection for all steps, resident in SBUF (bf16)
        gx_sb = wp.tile([128, MT * G], BF16)

        # per-lane state tiles (both lanes at base partition 0)
        hT = [None, None]  # [128, KC*BL], h^T packed, bf16
        for l in range(2):
            t0 = st.tile([128, KC * BL], BF16, tag=f"ht{l}", bufs=2)
            nc.sync.dma_start(out=t0[:], in_=ht0[l])
            hT[l] = t0

        cst, gt, fct, mt_, tch = [], [], [], [], []
        hst = [None, None]
        for l in range(2):
            cst.append(st.tile([BL, D], F32, name=f"cst{l}"))
            gt.append(st.tile([BL, G], F32, name=f"gt{l}"))
            fct.append(st.tile([BL, D], F32, name=f"fct{l}"))
            mt_.append(st.tile([BL, D], F32, name=f"mt{l}"))
            tch.append(st.tile([BL, D], F32, name=f"tch{l}"))
            hst[l] = st.tile([BL, D], F32, tag=f"hst{l}", bufs=2,
                             name=f"hst{l}")
            nc.sync.dma_start(out=cst[l][:], in_=c0[l])

        # persistent per-lane PSUM tiles: 2 x [128, 2048] fp32 = all 8 banks
        pbl0 = pp.tile([128, G], F32, tag="pbl0", name="pbl0")
        pbl1 = pp.tile([128, G], F32, tag="pbl1", name="pbl1")

        # ---- phase A: gather + transpose + batched input projection ----
        with tc.tile_pool(name="ph0", bufs=1) as p0:
            wih0_sb = p0.tile([128, KC * G], BF16)
            load_w(wih0_sb, w[0])
            bias0_bc = p0.tile([128, G], BF16)
            nc.sync.dma_start(out=bias0_bc[:], in_=bias0)

            for m in range(MT):
                idx_m = p0.tile([128, 1], I32, tag="idx", bufs=2)
                nc.sync.dma_start(out=idx_m[:], in_=tok[m * 128 : (m + 1) * 128, :])
                emb_m = p0.tile([128, D], BF16, tag="embrows", bufs=1)
                nc.gpsimd.indirect_dma_start(
                    out=emb_m[:],
                    out_offset=None,
                    in_=emb,
                    in_offset=bass.IndirectOffsetOnAxis(ap=idx_m[:, :1], axis=0),
                )
                # transpose [tb, d] -> [d, tb] per 128-chunk of d
                pbl0_bf = pbl0[:].bitcast(BF16)
                for c in range(KC):
                    nc.tensor.transpose(
                        out=pbl0_bf[:, c * 128 : (c + 1) * 128],
                        in_=emb_m[:, c * 128 : (c + 1) * 128],
                        identity=id_bf[:],
                    )
                embT_m = p0.tile([128, D], BF16, tag="embT", bufs=1)
                nc.vector.tensor_copy(out=embT_m[:], in_=pbl0_bf[:, 0:D])
                # batched input matmul for this M-tile (per-bank psum slots)
                for n in range(NB):
                    for c in range(KC):
                        nc.tensor.matmul(
                            out=pbl1[:, _nsl(n)],
                            lhsT=embT_m[:, c * 128 : (c + 1) * 128],
                            rhs=wih0_sb[:, c * G + n * 512 : c * G + (n + 1) * 512],
                            start=(c == 0),
                            stop=(c == KC - 1),
                        )
                    nc.vector.tensor_add(
                        out=gx_sb[:, m * G + n * 512 : m * G + (n + 1) * 512],
                        in0=pbl1[:, _nsl(n)],
                        in1=bias0_bc[:, _nsl(n)],
                    )

        # ---- phase B: recurrent loop ------------------------------------
        # Iteration t: lane 0 = layer-0 step t, lane 1 = layer-1 step t-1.
        # Lane-1's h transpose for step t-2 runs at the top of iteration t.

        def mm_group(pb, col0, stat, w_sb, n, start, stop):
            for c in range(KC):
                nc.tensor.matmul(
                    out=pb[0:BL, col0 : col0 + 512],
                    lhsT=stat[:, c * BL : (c + 1) * BL],
                    rhs=w_sb[:, c * G + n * 512 : c * G + (n + 1) * 512],
                    start=start and c == 0,
                    stop=stop and c == KC - 1,
                )

        def transpose_h(pb, src):
            for c in range(KC):
                nc.tensor.transpose(
                    out=pb[:, c * BL : (c + 1) * BL],
                    in_=src[:BL, c * 128 : (c + 1) * 128],
                    identity=id_sb[:BL, :BL],
                )

        def chain(l):
            # gate adds are emitted by the caller; this emits the per-lane
            # activation chain.
            eng = nc.vector
            nc.scalar.activation(out=gt[l][:, FI], in_=gt[l][:, FI],
                                 func=AFT.Sigmoid)
            nc.scalar.activation(out=gt[l][:, GSL], in_=gt[l][:, GSL],
                                 func=AFT.Sigmoid, scale=2.0)
            nc.scalar.activation(out=gt[l][:, OSL], in_=gt[l][:, OSL],
                                 func=AFT.Sigmoid)
            eng.tensor_mul(out=fct[l][:], in0=gt[l][:, _nsl(BANK_F)],
                           in1=cst[l][:])
            eng.tensor_sub(out=fct[l][:], in0=fct[l][:],
                           in1=gt[l][:, _nsl(BANK_I)])
            eng.scalar_tensor_tensor(
                out=mt_[l][:], in0=gt[l][:, GSL], scalar=2.0,
                in1=gt[l][:, _nsl(BANK_I)],
                op0=mybir.AluOpType.mult, op1=mybir.AluOpType.mult,
            )
            eng.tensor_add(out=cst[l][:], in0=fct[l][:], in1=mt_[l][:])
            # tanh(c) = 2*sigmoid(2c) - 1
            nc.scalar.activation(out=tch[l][:], in_=cst[l][:],
                                 func=AFT.Sigmoid, scale=2.0)
            eng.tensor_scalar(
                out=tch[l][:], in0=tch[l][:], scalar1=2.0, scalar2=-1.0,
                op0=mybir.AluOpType.mult, op1=mybir.AluOpType.add,
            )
            h_new = st.tile([BL, D], F32, tag=f"hst{l}", bufs=2,
                            name=f"hst{l}n")
            eng.tensor_mul(out=h_new[:], in0=gt[l][:, OSL], in1=tch[l][:])
            hst[l] = h_new

        for rep in range(REPS):
          for t in range(T + 1):
            last = t == T
            first = t == 0
            gxt = None
            if not last:
                # stage this step's input-projection gates: SBUF->SBUF DMA
                # from the resident gx tile (no HBM traffic)
                gxt = wk.tile([BL, G], BF16, tag="gxt", bufs=3)
                nc.sync.dma_start(
                    out=gxt[:],
                    in_=gx_sb[
                        (t % TPM) * BL : (t % TPM + 1) * BL,
                        (t // TPM) * G : (t // TPM + 1) * G,
                    ],
                )

            # deferred lane-1 transpose: h1[t-2] -> hT[1]
            if t >= 2:
                transpose_h(pbl1, hst[1])
                hT1n = st.tile([128, KC * BL], BF16, tag="ht1", bufs=2,
                               name="hT1n")
                nc.vector.tensor_copy(out=hT1n[:], in_=pbl1[:, 0 : KC * BL])
                hT[1] = hT1n

            # lane-0 matmuls (step t): h0[t-1] @ W_hh0
            if not last:
                for n in range(NB):
                    mm_group(pbl0, n * 512, hT[0], whh0_sb, n, True, True)
            # lane-1 matmuls (step t-1): h0[t-1] @ W_ih1 + h1[t-2] @ W_hh1
            if not first:
                for n in range(NB):
                    mm_group(pbl1, n * 512, hT[0], wih1_sb, n, True, False)
                    mm_group(pbl1, n * 512, hT[1], whh1_sb, n, False, True)

            # lane-0 chain + transpose (priority: gates next iteration)
            if not last:
                nc.vector.tensor_add(out=gt[0][:, FI], in0=pbl0[:BL, FI],
                                     in1=gxt[:, FI])
                nc.vector.tensor_add(out=gt[0][:, GSL], in0=pbl0[:BL, GSL],
                                     in1=gxt[:, GSL])
                nc.vector.tensor_add(out=gt[0][:, OSL], in0=pbl0[:BL, OSL],
                                     in1=gxt[:, OSL])
                chain(0)
                transpose_h(pbl0, hst[0])
                hT0n = st.tile([128, KC * BL], BF16, tag="ht0", bufs=2,
                               name="hT0n")
                nc.vector.tensor_copy(out=hT0n[:], in_=pbl0[:, 0 : KC * BL])
                hT[0] = hT0n

            # lane-1 chain (h-transpose deferred to next iteration)
            if not first:
                nc.vector.tensor_add(out=gt[1][:, FI], in0=pbl1[:BL, FI],
                                     in1=bias1_sb[:, FI])
                nc.vector.tensor_add(out=gt[1][:, GSL], in0=pbl1[:BL, GSL],
                                     in1=bias1_sb[:, GSL])
                nc.vector.tensor_add(out=gt[1][:, OSL], in0=pbl1[:BL, OSL],
                                     in1=bias1_sb[:, OSL])
                chain(1)
                nc.sync.dma_start(out=out[:, t - 1, :], in_=hst[1][:])


_NC_CACHE = {}


def _get_nc():
    if "nc" not in _NC_CACHE:
        _NC_CACHE["nc"] = _build()
    return _NC_CACHE["nc"]


def _make_in_maps(inputs):
    import ml_dtypes

    bf16 = ml_dtypes.bfloat16

    tokens = np.asarray(inputs["prev_tgt_tokens"])[:, :T].astype(np.int32)  # [B, T]
    emb = np.ascontiguousarray(np.asarray(inputs["emb"], dtype=np.float32))
    W_ih = np.asarray(inputs["W_ih"], dtype=np.float32)
    W_hh = np.asarray(inputs["W_hh"], dtype=np.float32)
    b_ih = np.asarray(inputs["b_ih"], dtype=np.float32)
    b_hh = np.asarray(inputs["b_hh"], dtype=np.float32)
    hiddens = np.asarray(inputs["hiddens"], dtype=np.float32)
    cells = np.asarray(inputs["cells"], dtype=np.float32)

    def permute_gates(a, axis):
        # PyTorch gate order [i, f, g, o] -> kernel bank order [f, i, g, o]
        blocks = np.split(a, 4, axis=axis)
        return np.concatenate([blocks[1], blocks[0], blocks[2], blocks[3]], axis=axis)

    emb_bf = np.ascontiguousarray(emb.astype(bf16))
    wih0t = np.ascontiguousarray(permute_gates(W_ih[0].T, 1).astype(bf16))  # [D, G]
    whh0t = np.ascontiguousarray(permute_gates(W_hh[0].T, 1).astype(bf16))
    wih1t = np.ascontiguousarray(permute_gates(W_ih[1].T, 1).astype(bf16))
    whh1t = np.ascontiguousarray(permute_gates(W_hh[1].T, 1).astype(bf16))
    bias_all = permute_gates(b_ih + b_hh, 1)  # [2, G] fp32
    bias0 = np.ascontiguousarray(
        np.broadcast_to(bias_all[0][None, :], (128, G)).astype(bf16)
    )
    bias1 = np.ascontiguousarray(
        np.broadcast_to(bias_all[1][None, :], (BL, G)).astype(np.float32)
    )

    in_maps = []
    for core in range(N_CORES):
        sl = slice(core * BL, (core + 1) * BL)
        tok_tm = np.ascontiguousarray(tokens[sl].T.reshape(BL * T, 1))  # t-major
        ht = np.empty((2, 128, KC * BL), dtype=np.float32)
        for l in range(2):
            # [BL, D] -> h^T [D, BL] -> [KC, 128, BL] -> [128, KC, BL]
            htl = hiddens[l, sl].T.reshape(KC, 128, BL).transpose(1, 0, 2)
            ht[l] = htl.reshape(128, KC * BL)
        cin = np.ascontiguousarray(cells[:, sl, :])
        in_maps.append(
            {
                "tokens": tok_tm,
                "emb": emb_bf,
                "wih0t": wih0t,
                "whh0t": whh0t,
                "wih1t": wih1t,
                "whh1t": whh1t,
                "bias0": bias0,
                "bias1": bias1,
                "ht_init": np.ascontiguousarray(ht.astype(bf16)),
                "c_init": cin,
            }
        )
    return in_maps


def run(inputs, trace=False, **kwargs):
    """Build (cached), run on 8 cores, return (full_output, BassKernelResults)."""
    nc = _get_nc()
    in_maps = _make_in_maps(inputs)
    res = run_bass_kernel_spmd(
        nc, in_maps, core_ids=list(range(N_CORES)), trace=trace, **kwargs
    )
    out = np.concatenate([r["out"] for r in res.results], axis=0)  # [B, T, D]
    return out, res


def kernel(**inputs) -> np.ndarray:
    out, _ = run(inputs, trace=False)
    return out


# revision 15
# speedup vs baseline: 1.2089x; 1.2089x over previous
"""Trainium2 Bass kernel: 2-layer LSTM decoder with embedding lookup.

Reference computation (per nn.Decoder):
    tgt_embed = emb[prev_tgt_tokens]                      # [B, T, D]
    for t in 0..T-1:
        x = tgt_embed[:, t]
        for l in 0..1:
            gates = x @ W_ih[l].T + b_ih[l] + h[l] @ W_hh[l].T + b_hh[l]
            i, f, g, o = split(gates, 4)
            c[l] = sigmoid(f) * c[l] + sigmoid(i) * tanh(g)
            h[l] = sigmoid(o) * tanh(c[l])
            x = h[l]
        out[:, t] = h[1]

Sharding: data-parallel over batch B=64 across 8 cores (8 rows each);
embedding + LSTM weights replicated; the sequential time loop runs
on-device per core, fully unrolled.

Kernel design (per core):
  - All matmul operands (emb rows, weights, h) are bf16, converted
    host-side so no on-device staging/convert copies are needed and the
    HBM weight traffic is halved. PSUM accumulation and the gate/state
    chain stay fp32.
  - Embedding gather via indirect DMA (128 rows per call), PE-transposed
    into K-major layout.
  - Input projection x @ W_ih[0].T batched over all T steps as one big
    matmul; the result stays resident in SBUF (bf16) instead of a DRAM
    round trip, and is staged per step with a small SBUF->SBUF DMA.
  - Recurrent loop: iteration t runs layer-0 step t and layer-1 step t-1.
    Each lane has its own PSUM tile and its own (unstacked) activation
    chain so the lane-0 recurrence - which gates the next iteration's
    matmuls - completes while the PE is still streaming lane-1 weights.
    The lane-1 h-transpose is deferred one further iteration so it never
    head-of-line blocks the PE queue.
  - All activations are Sigmoid (tanh(x) = 2*sigmoid(2x)-1) so the ACT
    engine never reloads its function table.
"""

import os

import numpy as np

import concourse.bass as bass
import concourse.mybir as mybir
import concourse.tile as tile
from concourse import bacc
from concourse.bass_utils import run_bass_kernel_spmd
from concourse.masks import make_identity

N_CORES = 8
B = 64
T = int(os.environ.get("BASS_LSTM_T", "128"))
D = 512
V = 32000
G = 4 * D            # 2048 gate dims per layer
BL = B // N_CORES    # 8 batch rows per core
KC = D // 128        # 4 contraction chunks of 128
NB = G // 512        # 4 PSUM banks of 512 per gate vector
MT = BL * T // 128   # M-tiles (128 token rows each) for the input matmul
TPM = 128 // BL      # time steps per M-tile (16)
REPS = int(os.environ.get("BASS_LSTM_REPS", "1"))  # timing-only: loop phase B
F32 = mybir.dt.float32
BF16 = mybir.dt.bfloat16
I32 = mybir.dt.int32
AFT = mybir.ActivationFunctionType

# gate banks after host-side permutation: [f, i, g, o], stacked on
# PSUM partition groups 0/32/64/96 via PE column-group tiling
BANK_F, BANK_I, BANK_G, BANK_O = 0, 1, 2, 3
RF, RI = slice(0, 8), slice(32, 40)
RG, RO = slice(64, 72), slice(96, 104)
RFI, RALL = slice(0, 40), slice(0, 104)


def _nsl(n):
    return slice(n * 512, (n + 1) * 512)


def _build():
    nc = bacc.Bacc(
        "TRN2",
        target_bir_lowering=False,
        debug=False,
        enable_asserts=False,
        num_devices=N_CORES,
    )

    tok_d = nc.dram_tensor("tokens", [BL * T, 1], I32, kind="ExternalInput")
    emb_d = nc.dram_tensor("emb", [V, D], BF16, kind="ExternalInput")
    wih0_d = nc.dram_tensor("wih0t", [D, G], BF16, kind="ExternalInput")
    whh0_d = nc.dram_tensor("whh0t", [D, G], BF16, kind="ExternalInput")
    wih1_d = nc.dram_tensor("wih1t", [D, G], BF16, kind="ExternalInput")
    whh1_d = nc.dram_tensor("whh1t", [D, G], BF16, kind="ExternalInput")
    bias0_d = nc.dram_tensor("bias0", [128, G], BF16, kind="ExternalInput")
    bias1_d = nc.dram_tensor("bias1", [128, 512], F32, kind="ExternalInput")
    ht_d = nc.dram_tensor("ht_init", [2, 128, KC * BL], BF16, kind="ExternalInput")
    c_d = nc.dram_tensor("c_init", [2, BL, D], F32, kind="ExternalInput")
    out_d = nc.dram_tensor("out", [BL, T, D], F32, kind="ExternalOutput")

    with tile.TileContext(nc) as tc:
        _body(
            tc,
            tok=tok_d.ap(),
            emb=emb_d.ap(),
            w=[wih0_d.ap(), whh0_d.ap(), wih1_d.ap(), whh1_d.ap()],
            bias0=bias0_d.ap(),
            bias1=bias1_d.ap(),
            ht0=ht_d.ap(),
            c0=c_d.ap(),
            out=out_d.ap(),
        )
    nc.compile()
    return nc


def _body(tc, tok, emb, w, bias0, bias1, ht0, c0, out):
    nc = tc.nc
    with (
        tc.tile_pool(name="wpool", bufs=1) as wp,
        tc.tile_pool(name="state", bufs=1) as st,
        tc.tile_pool(name="work", bufs=2) as wk,
        tc.tile_pool(name="pspool", bufs=1, space="PSUM") as pp,
    ):
        # ---- persistent tiles -------------------------------------------
        id_sb = wp.tile([128, 128], F32)
        make_identity(nc, id_sb[:])
        id_bf = wp.tile([128, 128], BF16)
        make_identity(nc, id_bf[:])

        whh0_sb = wp.tile([128, KC * G], BF16)
        wih1_sb = wp.tile([128, KC * G], BF16)
        whh1_sb = wp.tile([128, KC * G], BF16)

        def load_w(dst, src_ap):
            # one DMA: [D, G] viewed as [128, KC, G] chunk-major
            nc.sync.dma_start(
                out=dst[:].rearrange("p (c n) -> p c n", c=KC),
                in_=src_ap.rearrange("(c p) n -> p c n", p=128),
            )

        load_w(whh0_sb, w[1])
        load_w(wih1_sb, w[2])
        load_w(whh1_sb, w[3])

        bias1_sb = wp.tile([128, 512], F32)
        nc.sync.dma_start(out=bias1_sb[:], in_=bias1)

        # input projection for all steps, resident in SBUF (bf16)
        gx_sb = wp.tile([128, MT * G], BF16)

        # per-lane state tiles (both lanes at base partition 0)
        hT = [None, None]  # [128, KC*BL], h^T packed, bf16
        for l in range(2):
            t0 = st.tile([128, KC * BL], BF16, tag=f"ht{l}", bufs=2)
            nc.sync.dma_start(out=t0[:], in_=ht0[l])
            hT[l] = t0

        cst, gt, fct, mt_, tch = [], [], [], [], []
        sig_i, sig_g, sig_o = [], [], []
        hst = [None, None]
        for l in range(2):
            cst.append(st.tile([BL, D], F32, name=f"cst{l}"))
            gt.append(st.tile([128, 512], F32, name=f"gt{l}"))
            fct.append(st.tile([BL, D], F32, name=f"fct{l}"))
            mt_.append(st.tile([BL, D], F32, name=f"mt{l}"))
            tch.append(st.tile([BL, D], F32, name=f"tch{l}"))
            sig_i.append(st.tile([BL, D], F32, name=f"sig_i{l}"))
            sig_g.append(st.tile([BL, D], F32, name=f"sig_g{l}"))
            sig_o.append(st.tile([BL, D], F32, name=f"sig_o{l}"))
            hst[l] = st.tile([BL, D], F32, tag=f"hst{l}", bufs=2,
                             name=f"hst{l}")
            nc.vector.memset(gt[l][:], 0.0)
            nc.sync.dma_start(out=cst[l][:], in_=c0[l])

        # per-lane PSUM tiles (bank-stacked gates): 1 bank each, plus a
        # rotating transpose target and phase-A accumulators
        pbl0 = pp.tile([128, 512], F32, tag="pbl0", name="pbl0")
        pbl1 = pp.tile([128, 512], F32, tag="pbl1", name="pbl1")

        # ---- phase A: gather + transpose + batched input projection ----
        with tc.tile_pool(name="ph0", bufs=1) as p0:
            wih0_sb = p0.tile([128, KC * G], BF16)
            load_w(wih0_sb, w[0])
            bias0_bc = p0.tile([128, G], BF16)
            nc.sync.dma_start(out=bias0_bc[:], in_=bias0)

            for m in range(MT):
                idx_m = p0.tile([128, 1], I32, tag="idx", bufs=2)
                nc.sync.dma_start(out=idx_m[:], in_=tok[m * 128 : (m + 1) * 128, :])
                emb_m = p0.tile([128, D], BF16, tag="embrows", bufs=1)
                nc.gpsimd.indirect_dma_start(
                    out=emb_m[:],
                    out_offset=None,
                    in_=emb,
                    in_offset=bass.IndirectOffsetOnAxis(ap=idx_m[:, :1], axis=0),
                )
                # transpose [tb, d] -> [d, tb] per 128-chunk of d
                psT = pp.tile([128, 512], F32, tag="psA", bufs=2, name="psT")
                psT_bf = psT[:].bitcast(BF16)
                for c in range(KC):
                    nc.tensor.transpose(
                        out=psT_bf[:, c * 128 : (c + 1) * 128],
                        in_=emb_m[:, c * 128 : (c + 1) * 128],
                        identity=id_bf[:],
                    )
                embT_m = p0.tile([128, D], BF16, tag="embT", bufs=1)
                nc.vector.tensor_copy(out=embT_m[:], in_=psT_bf[:, 0:D])
                # batched input matmul for this M-tile (per-bank psum slots)
                for n in range(NB):
                    psm = pp.tile([128, 512], F32, tag="psA", bufs=2, name="psm")
                    for c in range(KC):
                        nc.tensor.matmul(
                            out=psm[:, :],
                            lhsT=embT_m[:, c * 128 : (c + 1) * 128],
                            rhs=wih0_sb[:, c * G + n * 512 : c * G + (n + 1) * 512],
                            start=(c == 0),
                            stop=(c == KC - 1),
                        )
                    nc.vector.tensor_add(
                        out=gx_sb[:, m * G + n * 512 : m * G + (n + 1) * 512],
                        in0=psm[:, :],
                        in1=bias0_bc[:, _nsl(n)],
                    )

        # ---- phase B: recurrent loop ------------------------------------
        # Iteration t: lane 0 = layer-0 step t, lane 1 = layer-1 step t-1.
        # Gate bank n lands on PSUM partitions 32n:32n+8 (PE column-group
        # tiling), so one DVE add + short ACT ops cover all four banks.
        # Lane-1's h transpose for step t-2 runs at the top of iteration t.

        def mm_group(pb, stat, w_sb, n, start, stop):
            for c in range(KC):
                nc.tensor.matmul(
                    out=pb[32 * n : 32 * n + BL, :],
                    lhsT=stat[:, c * BL : (c + 1) * BL],
                    rhs=w_sb[:, c * G + n * 512 : c * G + (n + 1) * 512],
                    start=start and c == 0,
                    stop=stop and c == KC - 1,
                    # auto-derive caps base_partition at 64; bank 3 sits at 96
                    tile_position=(0, 32 * n),
                )

        def transpose_h(pb, col0, src_t):
            for c in range(KC):
                nc.tensor.transpose(
                    out=pb[:, col0 + c * BL : col0 + (c + 1) * BL],
                    in_=src_t[:BL, c * 128 : (c + 1) * 128],
                    identity=id_sb[:BL, :BL],
                )

        def chain(l):
            # gate adds are emitted by the caller. The sigmoids relocate
            # the I/G/O gates from PSUM bank partition groups (32/64/96)
            # down to base-0 tiles: binary DVE ops need both SBUF inputs
            # at the same start partition.
            eng = nc.vector
            g = gt[l]
            nc.scalar.activation(out=g[RF, :], in_=g[RF, :], func=AFT.Sigmoid)
            nc.scalar.activation(out=sig_i[l][:], in_=g[RI, :], func=AFT.Sigmoid)
            nc.scalar.activation(out=sig_g[l][:], in_=g[RG, :],
                                 func=AFT.Sigmoid, scale=2.0)
            nc.scalar.activation(out=sig_o[l][:], in_=g[RO, :], func=AFT.Sigmoid)
            eng.tensor_mul(out=fct[l][:], in0=g[RF, :], in1=cst[l][:])
            eng.tensor_sub(out=fct[l][:], in0=fct[l][:], in1=sig_i[l][:])
            eng.scalar_tensor_tensor(
                out=mt_[l][:], in0=sig_g[l][:], scalar=2.0, in1=sig_i[l][:],
                op0=mybir.AluOpType.mult, op1=mybir.AluOpType.mult,
            )
            eng.tensor_add(out=cst[l][:], in0=fct[l][:], in1=mt_[l][:])
            nc.scalar.activation(out=tch[l][:], in_=cst[l][:],
                                 func=AFT.Sigmoid, scale=2.0)
            eng.tensor_scalar(
                out=tch[l][:], in0=tch[l][:], scalar1=2.0, scalar2=-1.0,
                op0=mybir.AluOpType.mult, op1=mybir.AluOpType.add,
            )
            h_new = st.tile([BL, D], F32, tag=f"hst{l}", bufs=2,
                            name=f"hst{l}n")
            eng.tensor_mul(out=h_new[:], in0=sig_o[l][:], in1=tch[l][:])
            hst[l] = h_new

        # pre-zero the rotating gxt slots once so the stacked gate add can
        # read the junk rows between the bank groups
        for _ in range(3):
            gz = wk.tile([128, 512], BF16, tag="gxt", bufs=3, name="gz")
            nc.vector.memset(gz[:], 0.0)

        for rep in range(REPS):
          for t in range(T + 1):
            last = t == T
            first = t == 0
            gxt = None
            if not last:
                # stage this step's input-projection gates: SBUF->SBUF DMA
                # from the resident gx tile, scattered to the bank groups
                gxt = wk.tile([128, 512], BF16, tag="gxt", bufs=3)
                for n in range(NB):
                    nc.sync.dma_start(
                        out=gxt[32 * n : 32 * n + BL, :],
                        in_=gx_sb[
                            (t % TPM) * BL : (t % TPM + 1) * BL,
                            (t // TPM) * G + n * 512
                            : (t // TPM) * G + (n + 1) * 512,
                        ],
                    )

            # rotating PE-transpose target (T0 cols 0:32, T1 cols 32:64)
            pT = pp.tile([128, 2 * KC * BL], F32, tag="pT", bufs=2, name="pT")

            # deferred lane-1 transpose: h1[t-2] -> hT[1]
            if t >= 2:
                transpose_h(pT, KC * BL, hst[1])
                hT1n = st.tile([128, KC * BL], BF16, tag="ht1", bufs=2,
                               name="hT1n")
                nc.vector.tensor_copy(
                    out=hT1n[:], in_=pT[:, KC * BL : 2 * KC * BL]
                )
                hT[1] = hT1n

            # lane-0 matmuls (step t): h0[t-1] @ W_hh0
            if not last:
                for n in range(NB):
                    mm_group(pbl0, hT[0], whh0_sb, n, True, True)
            # lane-1 matmuls (step t-1): h0[t-1] @ W_ih1 + h1[t-2] @ W_hh1
            if not first:
                for n in range(NB):
                    mm_group(pbl1, hT[0], wih1_sb, n, True, False)
                    mm_group(pbl1, hT[1], whh1_sb, n, False, True)

            # lane-0 chain + transpose (priority: gates next iteration)
            if not last:
                nc.vector.tensor_add(out=gt[0][RALL, :], in0=pbl0[RALL, :],
                                     in1=gxt[RALL, :])
                chain(0)
                transpose_h(pT, 0, hst[0])
                hT0n = st.tile([128, KC * BL], BF16, tag="ht0", bufs=2,
                               name="hT0n")
                nc.vector.tensor_copy(out=hT0n[:], in_=pT[:, 0 : KC * BL])
                hT[0] = hT0n

            # lane-1 chain (h-transpose deferred to next iteration)
            if not first:
                nc.vector.tensor_add(out=gt[1][RALL, :], in0=pbl1[RALL, :],
                                     in1=bias1_sb[RALL, :])
                chain(1)
                nc.sync.dma_start(out=out[:, t - 1, :], in_=hst[1][:])


_NC_CACHE = {}


def _get_nc():
    if "nc" not in _NC_CACHE:
        _NC_CACHE["nc"] = _build()
    return _NC_CACHE["nc"]


def _make_in_maps(inputs):
    import ml_dtypes

    bf16 = ml_dtypes.bfloat16

    tokens = np.asarray(inputs["prev_tgt_tokens"])[:, :T].astype(np.int32)  # [B, T]
    emb = np.ascontiguousarray(np.asarray(inputs["emb"], dtype=np.float32))
    W_ih = np.asarray(inputs["W_ih"], dtype=np.float32)
    W_hh = np.asarray(inputs["W_hh"], dtype=np.float32)
    b_ih = np.asarray(inputs["b_ih"], dtype=np.float32)
    b_hh = np.asarray(inputs["b_hh"], dtype=np.float32)
    hiddens = np.asarray(inputs["hiddens"], dtype=np.float32)
    cells = np.asarray(inputs["cells"], dtype=np.float32)

    def permute_gates(a, axis):
        # PyTorch gate order [i, f, g, o] -> kernel bank order [f, i, g, o]
        blocks = np.split(a, 4, axis=axis)
        return np.concatenate([blocks[1], blocks[0], blocks[2], blocks[3]], axis=axis)

    emb_bf = np.ascontiguousarray(emb.astype(bf16))
    wih0t = np.ascontiguousarray(permute_gates(W_ih[0].T, 1).astype(bf16))  # [D, G]
    whh0t = np.ascontiguousarray(permute_gates(W_hh[0].T, 1).astype(bf16))
    wih1t = np.ascontiguousarray(permute_gates(W_ih[1].T, 1).astype(bf16))
    whh1t = np.ascontiguousarray(permute_gates(W_hh[1].T, 1).astype(bf16))
    bias_all = permute_gates(b_ih + b_hh, 1)  # [2, G] fp32
    bias0 = np.ascontiguousarray(
        np.broadcast_to(bias_all[0][None, :], (128, G)).astype(bf16)
    )
    bias1 = np.zeros((128, 512), dtype=np.float32)
    for n in range(NB):
        bias1[32 * n : 32 * n + BL, :] = bias_all[1][n * 512 : (n + 1) * 512]

    in_maps = []
    for core in range(N_CORES):
        sl = slice(core * BL, (core + 1) * BL)
        tok_tm = np.ascontiguousarray(tokens[sl].T.reshape(BL * T, 1))  # t-major
        ht = np.empty((2, 128, KC * BL), dtype=np.float32)
        for l in range(2):
            # [BL, D] -> h^T [D, BL] -> [KC, 128, BL] -> [128, KC, BL]
            htl = hiddens[l, sl].T.reshape(KC, 128, BL).transpose(1, 0, 2)
            ht[l] = htl.reshape(128, KC * BL)
        cin = np.ascontiguousarray(cells[:, sl, :])
        in_maps.append(
            {
                "tokens": tok_tm,
                "emb": emb_bf,
                "wih0t": wih0t,
                "whh0t": whh0t,
                "wih1t": wih1t,
                "whh1t": whh1t,
                "bias0": bias0,
                "bias1": bias1,
                "ht_init": np.ascontiguousarray(ht.astype(bf16)),
                "c_init": cin,
            }
        )
    return in_maps


def run(inputs, trace=False, **kwargs):
    """Build (cached), run on 8 cores, return (full_output, BassKernelResults)."""
    nc = _get_nc()
    in_maps = _make_in_maps(inputs)
    res = run_bass_kernel_spmd(
        nc, in_maps, core_ids=list(range(N_CORES)), trace=trace, **kwargs
    )
    out = np.concatenate([r["out"] for r in res.results], axis=0)  # [B, T, D]
    return out, res


def kernel(**inputs) -> np.ndarray:
    out, _ = run(inputs, trace=False)
    return out
